# revision 33
# baseline (speedup 1.0000x reference)
"""MLA (multi-latent attention) prefill kernel for Trainium2, 8 NeuronCores.

Tensor-parallel over heads (2 heads/core) for q/kv_b/o; the shared kv_a
latent projection + rmsnorm + k-rope is sharded over the SEQUENCE dim
(256 positions/core) and the normalized latent is AllGathered through an
HBM bounce buffer, overlapped with the q projection. Per-core partial
o_proj outputs are summed on the host.

Attention uses a transposed-score layout: scoresT[k, q] tiles come
straight out of the PE (stationary = kT 128-col tile, moving = qT), exp
runs ACT psum->sbuf, and both the softmax denominator (ones-vector
matmul) and the PV product consume the bf16 expT tiles directly -- no
PE transposes and no psum->sbuf probability copies. Causal masking adds
one triangular [128,128] mask on diagonal tiles only, and diagonal
score/exp/PV work is narrowed to the valid q range.

Head 1 uses a swapped row layout (rope rows 0:64, nope rows 64:128) in
qT/kT so every DVE epilogue is partition-shift-free.

o_proj for superblock B is interleaved into superblock B+1's attention
stream to cover the softmax-normalize latency; outputs DMA from SBUF in
bf16 and the 8 partial results are summed on the host.
"""
import os
import sys
import types
import numpy as np
import ml_dtypes

import concourse.bass as bass
import concourse.mybir as mybir
import concourse.tile as tile
from concourse import bacc, bass_utils

F32 = mybir.dt.float32
BF16 = mybir.dt.bfloat16

S, HID = 2048, 2048
H, NOPE, ROPE, VD, KLR = 16, 64, 64, 128, 512
QD = NOPE + ROPE          # 128
SCALE = QD ** -0.5
EPS = 1e-6
NCORES = 8
HPC = H // NCORES         # heads per core = 2
SW = S // NCORES          # seq window per core = 256
HC = HID // 128           # 16 hid chunks
EXPB = 20.0               # fixed exp bias (overflow headroom)
NEG = -3.0e38


def build_nc():
    nc = bacc.Bacc("TRN2", target_bir_lowering=False, debug=False,
                   num_devices=NCORES)
    dr = {}
    dr["hT"] = nc.dram_tensor("hT", [HID, S], BF16, kind="ExternalInput")
    dr["hwin"] = nc.dram_tensor("hwin", [HID, SW], BF16, kind="ExternalInput")
    dr["wq"] = nc.dram_tensor("wq", [HID, 384], BF16, kind="ExternalInput")
    dr["wkva"] = nc.dram_tensor("wkva", [HID, 640], BF16, kind="ExternalInput")
    dr["wkvb"] = nc.dram_tensor("wkvb", [KLR, 384], BF16, kind="ExternalInput")
    dr["wo"] = nc.dram_tensor("wo", [HPC * VD, HID], BF16, kind="ExternalInput")
    dr["cosd"] = nc.dram_tensor("cosd", [128, S], BF16, kind="ExternalInput")
    dr["sind"] = nc.dram_tensor("sind", [128, S], BF16, kind="ExternalInput")
    dr["cwin"] = nc.dram_tensor("cwin", [128, SW], BF16, kind="ExternalInput")
    dr["swin"] = nc.dram_tensor("swin", [128, SW], BF16, kind="ExternalInput")
    dr["o"] = nc.dram_tensor("o", [S, HID], BF16, kind="ExternalOutput")

    with tile.TileContext(nc) as tc:
        build_tile_kernel(nc, tc, {k: v.ap() for k, v in dr.items()})
    nc.compile()
    return nc


def build_tile_kernel(nc, tc, d):
    from contextlib import ExitStack
    with ExitStack() as ctx:
        _build_tile_kernel(nc, tc, d, ctx)


def _build_tile_kernel(nc, tc, d, ctx):
    AF = mybir.ActivationFunctionType
    ALU = mybir.AluOpType

    consts = ctx.enter_context(tc.tile_pool(name="consts", bufs=1))
    big = ctx.enter_context(tc.tile_pool(name="big", bufs=1))
    work = ctx.enter_context(tc.tile_pool(name="work", bufs=2))
    htp = ctx.enter_context(tc.tile_pool(name="htp", bufs=3))
    outp = ctx.enter_context(tc.tile_pool(name="outp", bufs=2))
    ps = ctx.enter_context(tc.tile_pool(name="ps", bufs=2, space="PSUM"))
    dram = ctx.enter_context(tc.tile_pool(name="dram", bufs=1, space="DRAM"))

    # ---- input DMAs ------------------------------------------------------
    # Pre-trigger DMA is kept to the phase-A minimum (wkva+hwin+cwin/swin,
    # 3.7MB): all 8 cores share HBM bandwidth, and every extra early byte
    # delays the slowest core's AllGather trigger, which gates everyone.
    # The remaining input DMAs ride the scalar queue BEHIND the bounce-in
    # writes, so they only start flowing once the collective is in flight.
    wkva_sb = consts.tile([128, HC, 640], BF16)
    wkva_r = d["wkva"].rearrange("(k p) m -> p k m", p=128)
    hwin_sb = consts.tile([128, HC, SW], BF16)
    hwin_r = d["hwin"].rearrange("(k p) m -> p k m", p=128)
    for k in range(HC):
        nc.sync.dma_start(out=wkva_sb[:, k, :], in_=wkva_r[:, k, :])
        if k % 2 == 0:
            nc.sync.dma_start(out=hwin_sb[:, k:k + 2, :],
                              in_=hwin_r[:, k:k + 2, :])
    cwin_sb = consts.tile([128, SW], BF16)
    nc.sync.dma_start(out=cwin_sb[:], in_=d["cwin"])
    swin_sb = consts.tile([128, SW], BF16)
    nc.sync.dma_start(out=swin_sb[:], in_=d["swin"])
    # tiles allocated now, DMAs emitted after the bounce-in writes
    wq_sb = consts.tile([128, HC, 384], BF16)
    ht01 = [htp.tile([128, HC, 512], BF16, tag="ht", bufs=2, name=f"ht{c}")
            for c in range(2)]
    cos_sb = consts.tile([128, S], BF16)
    sin_sb = consts.tile([128, S], BF16)
    wkvb_sb = consts.tile([128, 4, 384], BF16)
    wo_sb = consts.tile([128, HPC, HID], BF16)

    # ---- small constants -------------------------------------------------
    ones_sb = consts.tile([128, 1], BF16)
    nc.vector.memset(ones_sb[:], 1.0)
    ones_row = consts.tile([1, 128], BF16)
    nc.vector.memset(ones_row[:], 1.0)
    nexpb = consts.tile([128, 1], F32)
    nc.vector.memset(nexpb[:], -EXPB)
    epsb = consts.tile([1, 1], F32)
    nc.vector.memset(epsb[:], EPS)
    # triangular causal mask for diagonal 128x128 tiles of scoresT[k, q]:
    # keep (0.0) where q_local >= k_local, else NEG
    trimask = consts.tile([128, 128], F32)
    nc.gpsimd.memset(trimask[:], 0.0)
    nc.gpsimd.affine_select(out=trimask[:], in_=trimask[:],
                            compare_op=ALU.is_ge, fill=NEG, base=0,
                            pattern=[[1, 128]], channel_multiplier=-1)

    # ---- persistent activations -----------------------------------------
    # head 1 uses swapped halves: rope rows 0:64, nope rows 64:128
    qT = [big.tile([128, S], BF16, tag=f"qT{h}", name=f"qT{h}")
          for h in range(HPC)]
    kT = [big.tile([128, S], BF16, tag=f"kT{h}", name=f"kT{h}")
          for h in range(HPC)]
    v_sb = big.tile([128, S // 128, HPC * VD], BF16, tag="v")
    ckv_sb = big.tile([128, 4, S], BF16, tag="ckv")

    # =====================================================================
    # PHASE A: sharded kv_a projection [640, SW] + rmsnorm + k-rope,
    # AllGather of the normalized latent + roped k_pe across 8 cores.
    # =====================================================================
    pk_tags = ["sc", "pv", "dn", "po", "sc"]
    pk = [ps.tile([128, SW], F32, tag=pk_tags[m], name=f"pk{m}")
          for m in range(5)]
    sqt = work.tile([128, 4, SW], BF16, tag="sq", bufs=1)
    stg = big.tile([128, 5, SW], BF16, tag="stg")
    # k-major: step k consumes wkva/hwin chunk k right after its DMA lands
    for k in range(HC):
        for m in range(5):
            nc.tensor.matmul(pk[m][:], wkva_sb[:, k, m * 128:(m + 1) * 128],
                             hwin_sb[:, k, :], start=(k == 0),
                             stop=(k == HC - 1))
    for m in range(4):
        nc.scalar.activation(sqt[:, m, :], pk[m][:], AF.Square)
        nc.vector.tensor_copy(stg[:, m, :], pk[m][:])
    # k_pe rope: stg[0:64,4] = x'*cos + y*sin  (x' rows 0:64, y 64:128)
    t2a = work.tile([64, SW], F32, tag="t2a", bufs=1)
    nc.vector.tensor_tensor(t2a[:], pk[4][64:128, :],
                            swin_sb[64:128, :], ALU.mult)
    nc.vector.tensor_tensor(stg[0:64, 4, :], pk[4][0:64, :],
                            cwin_sb[0:64, :], ALU.mult)
    nc.vector.tensor_tensor(stg[0:64, 4, :], stg[0:64, 4, :],
                            t2a[:], ALU.add)

    # rms statistics: sum of squares over the 512 latent rows via PE
    ssum = ps.tile([1, SW], F32, tag="dn", name="ssum")
    for m in range(4):
        nc.tensor.matmul(ssum[:], ones_sb[:], sqt[:, m, :], start=(m == 0),
                         stop=(m == 3))
    rsq = work.tile([1, SW], F32, tag="rsq", bufs=1)
    nc.scalar.activation(rsq[:], ssum[:], AF.Sqrt, bias=epsb[:],
                         scale=1.0 / KLR)
    # preload the EXP table now -- the ACT queue stalls on the bounce /
    # readback DMA deps below, and phase D needs the table immediately
    wtile = work.tile([1, 1], F32, tag="wtile", bufs=1)
    nc.scalar.activation(wtile[:], epsb[:], AF.Exp)
    rs = work.tile([1, SW], F32, tag="rs", bufs=1)
    nc.vector.reciprocal(rs[:], rsq[:])
    rs_bf = work.tile([1, SW], BF16, tag="rs_bf", bufs=1)
    nc.vector.tensor_copy(rs_bf[:], rs[:])

    # bounce to DRAM (UNNORMALIZED latent + roped kpe + the rmsnorm scale
    # row), AllGather, read back.  Normalization is applied post-gather so
    # the big latent rows can ship while the stats chain still runs.  The
    # bounce/readback DMAs ride the scalar HWDGE queue: everything behind
    # them there (v copies, exps) is needed only after the gather anyway.
    bin_ = dram.tile([577, SW], BF16, tag="bin")
    bout = dram.tile([NCORES, 577, SW], BF16, tag="bout", addr_space="Shared")
    for m in range(4):
        nc.scalar.dma_start(out=bin_[m * 128:(m + 1) * 128, :],
                            in_=stg[:, m, :])
    nc.scalar.dma_start(out=bin_[512:576, :], in_=stg[0:64, 4, :])
    nc.scalar.dma_start(out=bin_[576:577, :], in_=rs_bf[:])
    nc.gpsimd.collective_compute(
        "AllGather", ALU.bypass,
        replica_groups=[list(range(NCORES))],
        ins=[bin_[:].opt()], outs=[bout[:].opt()])
    # deferred B-feed weight/activation DMAs (scalar queue, behind bounce)
    wq_r = d["wq"].rearrange("(k p) m -> p k m", p=128)
    for kp in range(4):
        nc.scalar.dma_start(out=wq_sb[:, 4 * kp:4 * kp + 4, :],
                            in_=wq_r[:, 4 * kp:4 * kp + 4, :])
    for c in range(2):
        for kp in range(HC // 2):
            nc.scalar.dma_start(
                out=ht01[c][:, 2 * kp:2 * kp + 2, :],
                in_=d["hT"][256 * kp:256 * (kp + 1),
                            c * 512:(c + 1) * 512].rearrange(
                    "(k p) m -> p k m", p=128))
    for i in range(2):
        cs = slice(i * 1024, (i + 1) * 1024)
        nc.scalar.dma_start(out=cos_sb[:, cs], in_=d["cosd"][:, cs])
        nc.scalar.dma_start(out=sin_sb[:, cs], in_=d["sind"][:, cs])
    nc.scalar.dma_start(out=wkvb_sb[:],
                        in_=d["wkvb"].rearrange("(k p) m -> p k m", p=128))
    wo_r = d["wo"].rearrange("(h p) n -> p h n", p=128)
    for h in range(HPC):
        nc.scalar.dma_start(out=wo_sb[:, h, :], in_=wo_r[:, h, :])
    # readback: scale row first (feeds the PE broadcast), then latent tiles
    rs_all = work.tile([1, S], BF16, tag="rs_all", bufs=1)
    nc.scalar.dma_start(
        out=rs_all[:].rearrange("p (c m) -> p c m", m=SW),
        in_=bout[:, 576:577, :].rearrange("c p m -> p c m"))
    rnorm = work.tile([128, 4, 512], BF16, tag="rnorm", bufs=1)
    for i in range(4):
        rn_ps = ps.tile([128, 512], F32, tag="po", name=f"rn_ps{i}")
        nc.tensor.matmul(rn_ps[:], ones_row[:],
                         rs_all[0:1, i * 512:(i + 1) * 512],
                         start=True, stop=True)
        nc.vector.tensor_copy(rnorm[:, i, :], rn_ps[:])
    for half in range(2):
        hs = slice(half * (NCORES // 2), (half + 1) * (NCORES // 2))
        for t in range(4):
            src = bout[:, t * 128:(t + 1) * 128, :]
            nc.scalar.dma_start(
                out=ckv_sb[:, t, half * (S // 2):(half + 1) * (S // 2)]
                    .rearrange("p (c m) -> p c m", m=SW),
                in_=src[hs].rearrange("c p m -> p c m"))
    rope_src = bout[:, 512:576, :].rearrange("c p m -> p c m")
    nc.scalar.dma_start(
        out=kT[0][64:128, :].rearrange("p (c m) -> p c m", m=SW),
        in_=rope_src)
    nc.scalar.dma_start(
        out=kT[1][0:64, :].rearrange("p (c m) -> p c m", m=SW),
        in_=rope_src)
    # normalize the gathered latent in place (per-seq rmsnorm scale)
    rn_flat = bass.AP(tensor=rnorm.tensor, offset=rnorm.offset,
                      ap=[rnorm.ap[0], [1, S]])
    for t in range(4):
        nc.vector.tensor_tensor(ckv_sb[:, t, :], ckv_sb[:, t, :],
                                rn_flat, ALU.mult)

    # =====================================================================
    # PHASE B: q projection (3 blocks: [nope|x']_h0, [x'|nope]_h1,
    # [y_h1|y_h0]) + rope epilogues.
    # =====================================================================
    pq_tags = ["sc", "pv", "dn"]
    for c in range(4):
        cs = slice(c * 512, (c + 1) * 512)
        if c < 2:
            ht_r = ht01[c]
        else:
            ht_r = htp.tile([128, HC, 512], BF16, tag="ht", bufs=2,
                            name=f"ht{c}")
            for kp in range(HC // 2):
                nc.sync.dma_start(
                    out=ht_r[:, 2 * kp:2 * kp + 2, :],
                    in_=d["hT"][256 * kp:256 * (kp + 1), cs].rearrange(
                        "(k p) m -> p k m", p=128))
        pq = [ps.tile([128, 512], F32, tag=pq_tags[m], name=f"pq{m}_{c}")
              for m in range(3)]
        # k-major: consume ht chunk k as soon as its DMA lands
        for k in range(HC):
            for m in range(3):
                nc.tensor.matmul(pq[m][:], wq_sb[:, k, m * 128:(m + 1) * 128],
                                 ht_r[:, k, :], start=(k == 0),
                                 stop=(k == HC - 1))
        t2 = work.tile([128, 512], F32, tag="t2", name=f"t2_{c}")
        # h0: nope rows 0:64, rope rows 64:128
        nc.vector.tensor_copy(qT[0][0:64, cs], pq[0][0:64, :])
        nc.vector.tensor_tensor(qT[0][64:128, cs], pq[0][64:128, :],
                                cos_sb[64:128, cs], ALU.mult)
        nc.vector.tensor_tensor(t2[64:128, :], pq[2][64:128, :],
                                sin_sb[64:128, cs], ALU.mult)
        nc.vector.tensor_tensor(qT[0][64:128, cs], qT[0][64:128, cs],
                                t2[64:128, :], ALU.add)
        # h1 (swapped): rope rows 0:64, nope rows 64:128
        nc.vector.tensor_copy(qT[1][64:128, cs], pq[1][64:128, :])
        nc.vector.tensor_tensor(qT[1][0:64, cs], pq[1][0:64, :],
                                cos_sb[0:64, cs], ALU.mult)
        nc.vector.tensor_tensor(t2[0:64, :], pq[2][0:64, :],
                                sin_sb[0:64, cs], ALU.mult)
        nc.vector.tensor_tensor(qT[1][0:64, cs], qT[1][0:64, cs],
                                t2[0:64, :], ALU.add)

    # =====================================================================
    # PHASE C: kv_b -- k_nope column-major into kT, v row-major into v_sb.
    # =====================================================================
    for c in range(4):
        cs = slice(c * 512, (c + 1) * 512)
        pn = ps.tile([128, 512], F32, tag="sc", name=f"pn{c}")
        for t in range(4):
            nc.tensor.matmul(pn[:], wkvb_sb[:, t, 0:128], ckv_sb[:, t, cs],
                             start=(t == 0), stop=(t == 3))
        nc.vector.tensor_copy(kT[0][0:64, cs], pn[0:64, :])
        nc.vector.tensor_copy(kT[1][64:128, cs], pn[64:128, :])
    for s16 in range(16):
        pvv = ps.tile([128, HPC * VD], F32, tag="pv", name=f"pvv{s16}")
        for t in range(4):
            nc.tensor.matmul(pvv[:], ckv_sb[:, t, s16 * 128:(s16 + 1) * 128],
                             wkvb_sb[:, t, 128:384], start=(t == 0),
                             stop=(t == 3))
        nc.scalar.copy(v_sb[:, s16, :], pvv[:])

    # =====================================================================
    # PHASE D: attention superblocks (512 queries each) + interleaved
    # o_proj of the previous superblock.
    # =====================================================================
    oproj_jobs = []

    def emit_oproj_job():
        if oproj_jobs:
            oproj_jobs.pop(0)()

    def make_oproj_jobs(Bq, a0, a1):
        jobs = []
        for t in range(4):
            ot = outp.tile([128, 4, 512], BF16, tag="ot", bufs=2,
                           name=f"ot{Bq}_{t}")
            for n in range(4):
                def job(Bq=Bq, t=t, n=n, ot=ot, a0=a0, a1=a1):
                    po = ps.tile([128, 512], F32, tag="po",
                                 name=f"po{Bq}_{t}_{n}")
                    nc.tensor.matmul(po[:], a0[:, t * 128:(t + 1) * 128],
                                     wo_sb[:, 0, n * 512:(n + 1) * 512],
                                     start=True, stop=False)
                    nc.tensor.matmul(po[:], a1[:, t * 128:(t + 1) * 128],
                                     wo_sb[:, 1, n * 512:(n + 1) * 512],
                                     start=False, stop=True)
                    if n % 2 == 0:
                        nc.vector.tensor_copy(ot[:, n, :], po[:])
                    else:
                        nc.scalar.copy(ot[:, n, :], po[:])
                    if n == 3:
                        nc.sync.dma_start(
                            out=d["o"][(4 * Bq + t) * 128:
                                       (4 * Bq + t + 1) * 128, :],
                            in_=ot[:])
                jobs.append(job)
        return jobs

    at_tiles = [None, None]
    for Bq in range(4):
        nkt = 4 * (Bq + 1)
        for h in range(HPC):
            pa = ps.tile([128, 512], F32, tag="pv", name=f"pa{Bq}_{h}")
            dnm = ps.tile([1, 512], F32, tag="dn", name=f"dn{Bq}_{h}")

            def emit_score(kt, h=h, Bq=Bq):
                j = kt - 4 * Bq
                off = j * 128 if j >= 0 else 0
                w = 512 - off
                sct = ps.tile([128, 512], F32, tag="sc",
                              name=f"sc{Bq}_{h}_{kt}")
                nc.tensor.matmul(
                    sct[:, 0:w], kT[h][:, kt * 128:(kt + 1) * 128],
                    qT[h][:, Bq * 512 + off:(Bq + 1) * 512],
                    start=True, stop=True)
                return sct, off, w

            cur = emit_score(0)
            for kt in range(nkt):
                sct, off, w = cur
                if kt + 1 < nkt:
                    cur = emit_score(kt + 1)
                if kt - 4 * Bq >= 0:  # diagonal tile: triangular mask
                    nc.vector.tensor_tensor(sct[:, 0:128], sct[:, 0:128],
                                            trimask[:], ALU.add)
                ex = work.tile([128, 512], BF16, tag="expT", bufs=3,
                               name=f"ex{Bq}_{h}_{kt}")
                nc.scalar.activation(ex[:, off:512], sct[:, 0:w], AF.Exp,
                                     bias=nexpb[:], scale=1.0)
                if h == 1:
                    # pop o_proj(Bq-1) work only in the h1 loop: by then the
                    # previous superblock's softmax-normalize chain is done.
                    # Popped between exp and ones/PV emission: the job's
                    # matmuls give the PE filler while exp(kt) is in flight,
                    # and the job's copies queue on DVE after the mask.
                    emit_oproj_job()
                    emit_oproj_job()
                nc.tensor.matmul(dnm[0:1, off:512], ones_sb[:],
                                 ex[:, off:512], start=(kt == 0),
                                 stop=(kt == nkt - 1))
                nc.tensor.matmul(pa[:, off:512],
                                 v_sb[:, kt, h * VD:(h + 1) * VD],
                                 ex[:, off:512], start=(kt == 0),
                                 stop=(kt == nkt - 1))

            rr = work.tile([1, 512], F32, tag="rr", name=f"rr{Bq}_{h}")
            nc.vector.reciprocal(rr[:], dnm[:])
            rr_bf = work.tile([1, 512], BF16, tag="rr_bf",
                              name=f"rr_bf{Bq}_{h}")
            nc.vector.tensor_copy(rr_bf[:], rr[:])
            rb_ps = ps.tile([128, 512], F32, tag="po", name=f"rb_ps{Bq}_{h}")
            nc.tensor.matmul(rb_ps[:], ones_row[:], rr_bf[:],
                             start=True, stop=True)
            rbc2 = work.tile([128, 512], BF16, tag="rbc2",
                             name=f"rbc2{Bq}_{h}")
            nc.vector.tensor_copy(rbc2[:], rb_ps[:])
            at = work.tile([128, 512], BF16, tag=f"at{h}", bufs=2,
                           name=f"at{Bq}_{h}")
            nc.vector.tensor_tensor(at[:], pa[:], rbc2[:], ALU.mult)
            at_tiles[h] = at
        oproj_jobs.extend(make_oproj_jobs(Bq, at_tiles[0], at_tiles[1]))
    while oproj_jobs:
        oproj_jobs.pop(0)()


# =========================================================================
# host side
# =========================================================================
_perm1 = np.concatenate([np.arange(0, ROPE, 2), np.arange(1, ROPE, 2)])
_perm2 = np.concatenate([np.arange(1, ROPE, 2), np.arange(0, ROPE, 2)])
_sgn2 = np.concatenate([-np.ones(ROPE // 2), np.ones(ROPE // 2)]).astype(np.float32)


def _host_prep(inputs):
    hidden = np.ascontiguousarray(np.asarray(inputs["hidden_states"],
                                             dtype=np.float32)[0])
    cos = np.asarray(inputs["cos"], dtype=np.float32)[0]
    sin = np.asarray(inputs["sin"], dtype=np.float32)[0]
    w_q = np.asarray(inputs["w_q"], dtype=np.float32)
    w_kv_a = np.asarray(inputs["w_kv_a"], dtype=np.float32)
    ln_w = np.asarray(inputs["kv_a_ln_w"], dtype=np.float32)
    w_kv_b = np.asarray(inputs["w_kv_b"], dtype=np.float32)
    w_o = np.asarray(inputs["w_o"], dtype=np.float32)

    hT = np.ascontiguousarray(hidden.T)
    cosT, sinT = cos.T, sin.T
    cosd = np.ascontiguousarray(np.concatenate([cosT, cosT], axis=0))
    sind = np.ascontiguousarray(np.concatenate([sinT, sinT], axis=0))

    kpe_cols = w_kv_a[:, KLR:]
    wkva_mod = np.ascontiguousarray(np.concatenate(
        [w_kv_a[:, :KLR], kpe_cols[:, _perm1], kpe_cols[:, _perm2] * _sgn2[None, :]],
        axis=1))
    wkvb_all = w_kv_b * ln_w[:, None]

    bf = ml_dtypes.bfloat16
    hT_bf = hT.astype(bf)
    wkva_bf = wkva_mod.astype(bf)
    cosd_bf = cosd.astype(bf)
    sind_bf = sind.astype(bf)

    in_maps = []
    for c in range(NCORES):
        heads = [HPC * c + i for i in range(HPC)]
        win = slice(c * SW, (c + 1) * SW)
        h0, h1 = heads
        wq_h0 = w_q[:, h0 * QD:(h0 + 1) * QD]
        wq_h1 = w_q[:, h1 * QD:(h1 + 1) * QD]
        b0 = np.concatenate([wq_h0[:, :NOPE], wq_h0[:, NOPE:][:, _perm1]],
                            axis=1)
        b1 = np.concatenate([wq_h1[:, NOPE:][:, _perm1], wq_h1[:, :NOPE]],
                            axis=1)
        b2 = np.concatenate([wq_h1[:, NOPE:][:, _perm2] * _sgn2[None, :],
                             wq_h0[:, NOPE:][:, _perm2] * _sgn2[None, :]],
                            axis=1)
        wq_mod = np.ascontiguousarray(
            np.concatenate([b0, b1, b2], axis=1) * SCALE)

        nope_b = [wkvb_all[:, h * (NOPE + VD):h * (NOPE + VD) + NOPE]
                  for h in heads]
        v_b = [wkvb_all[:, h * (NOPE + VD) + NOPE:(h + 1) * (NOPE + VD)]
               for h in heads]
        wkvb_mod = np.ascontiguousarray(np.concatenate(nope_b + v_b, axis=1))

        wo_mod = np.ascontiguousarray(w_o[h0 * VD:(h1 + 1) * VD, :])

        cwin = np.ascontiguousarray(np.concatenate(
            [cosT[:, win], cosT[:, win]], axis=0))
        swin = np.ascontiguousarray(np.concatenate(
            [sinT[:, win], sinT[:, win]], axis=0))

        in_maps.append({
            "hT": hT_bf,
            "hwin": np.ascontiguousarray(hT[:, win]).astype(bf),
            "wq": wq_mod.astype(bf),
            "wkva": wkva_bf,
            "wkvb": wkvb_mod.astype(bf),
            "wo": wo_mod.astype(bf),
            "cosd": cosd_bf, "sind": sind_bf,
            "cwin": cwin.astype(bf), "swin": swin.astype(bf),
        })
    return in_maps


def _install_ntff_hook():
    """Make trace=True work under axon (antenv.axon_hooks is absent in this
    image; back it with trn_agent_boot's ctypes hook)."""
    try:
        import antenv
        if "antenv.axon_hooks" in sys.modules:
            return
        from trn_agent_boot.trn_boot import _ntff_profile_via_ctypes
        hook = _ntff_profile_via_ctypes("/opt/axon/libaxon_pjrt.so")
        mod = types.ModuleType("antenv.axon_hooks")
        mod.get_axon_ntff_profile_hook = lambda: hook
        mod.set_axon_ntff_profile_hook = lambda h: None
        sys.modules["antenv.axon_hooks"] = mod
        antenv.axon_hooks = mod
    except Exception:
        pass


_nc_cache = None
last_results = None


def kernel(**inputs):
    global _nc_cache, last_results
    _install_ntff_hook()
    if _nc_cache is None:
        _nc_cache = build_nc()
    in_maps = _host_prep(inputs)
    trace = bool(os.environ.get("BASS_TRACE"))
    res = bass_utils.run_bass_kernel_spmd(
        _nc_cache, in_maps, core_ids=list(range(NCORES)), trace=trace)
    last_results = res
    total = res.results[0]["o"].astype(np.float32)
    for c in range(1, NCORES):
        total = total + res.results[c]["o"]
    return total.reshape(1, S, HID)


# revision 41
# speedup vs baseline: 1.0639x; 1.0639x over previous
"""MLA (multi-latent attention) prefill kernel for Trainium2, 8 NeuronCores.

Tensor-parallel over heads (2 heads/core) for q/kv_b/o; the shared kv_a
latent projection + rmsnorm + k-rope is sharded over the SEQUENCE dim
(256 positions/core) and the normalized latent is AllGathered through an
HBM bounce buffer, overlapped with the q projection. Per-core partial
o_proj outputs are summed on the host.

Attention uses a transposed-score layout: scoresT[k, q] tiles come
straight out of the PE (stationary = kT 128-col tile, moving = qT), exp
runs ACT psum->sbuf, and both the softmax denominator (ones-vector
matmul) and the PV product consume the bf16 expT tiles directly -- no
PE transposes and no psum->sbuf probability copies. Causal masking adds
one triangular [128,128] mask on diagonal tiles only, and diagonal
score/exp/PV work is narrowed to the valid q range.

Head 1 uses a swapped row layout (rope rows 0:64, nope rows 64:128) in
qT/kT so every DVE epilogue is partition-shift-free.

o_proj for superblock B is interleaved into superblock B+1's attention
stream to cover the softmax-normalize latency; outputs DMA from SBUF in
bf16 and the 8 partial results are summed on the host.
"""
import os
import sys
import types
import numpy as np
import ml_dtypes

import concourse.bass as bass
import concourse.mybir as mybir
import concourse.tile as tile
from concourse import bacc, bass_utils

F32 = mybir.dt.float32
BF16 = mybir.dt.bfloat16

S, HID = 2048, 2048
H, NOPE, ROPE, VD, KLR = 16, 64, 64, 128, 512
QD = NOPE + ROPE          # 128
SCALE = QD ** -0.5
EPS = 1e-6
NCORES = 8
HPC = H // NCORES         # heads per core = 2
SW = S // NCORES          # seq window per core = 256
HC = HID // 128           # 16 hid chunks
EXPB = 20.0               # fixed exp bias (overflow headroom)
NEG = -3.0e38


def build_nc():
    nc = bacc.Bacc("TRN2", target_bir_lowering=False, debug=False,
                   num_devices=NCORES)
    dr = {}
    dr["hT"] = nc.dram_tensor("hT", [HID, S], BF16, kind="ExternalInput")
    dr["hwin"] = nc.dram_tensor("hwin", [HID, SW], BF16, kind="ExternalInput")
    dr["wq"] = nc.dram_tensor("wq", [HID, 384], BF16, kind="ExternalInput")
    dr["wkva"] = nc.dram_tensor("wkva", [HID, 640], BF16, kind="ExternalInput")
    dr["wkvb"] = nc.dram_tensor("wkvb", [KLR, 384], BF16, kind="ExternalInput")
    dr["wo"] = nc.dram_tensor("wo", [HPC * VD, HID], BF16, kind="ExternalInput")
    dr["cosd"] = nc.dram_tensor("cosd", [128, S], BF16, kind="ExternalInput")
    dr["sind"] = nc.dram_tensor("sind", [128, S], BF16, kind="ExternalInput")
    dr["cwin"] = nc.dram_tensor("cwin", [128, SW], BF16, kind="ExternalInput")
    dr["swin"] = nc.dram_tensor("swin", [128, SW], BF16, kind="ExternalInput")
    dr["o"] = nc.dram_tensor("o", [S, HID], BF16, kind="ExternalOutput")

    with tile.TileContext(nc) as tc:
        build_tile_kernel(nc, tc, {k: v.ap() for k, v in dr.items()})
    nc.compile()
    return nc


def build_tile_kernel(nc, tc, d):
    from contextlib import ExitStack
    with ExitStack() as ctx:
        _build_tile_kernel(nc, tc, d, ctx)


def _build_tile_kernel(nc, tc, d, ctx):
    AF = mybir.ActivationFunctionType
    ALU = mybir.AluOpType

    consts = ctx.enter_context(tc.tile_pool(name="consts", bufs=1))
    big = ctx.enter_context(tc.tile_pool(name="big", bufs=1))
    work = ctx.enter_context(tc.tile_pool(name="work", bufs=2))
    htp = ctx.enter_context(tc.tile_pool(name="htp", bufs=3))
    outp = ctx.enter_context(tc.tile_pool(name="outp", bufs=2))
    ps = ctx.enter_context(tc.tile_pool(name="ps", bufs=2, space="PSUM"))
    dram = ctx.enter_context(tc.tile_pool(name="dram", bufs=1, space="DRAM"))

    # ---- input DMAs ------------------------------------------------------
    # Pre-trigger DMA is kept to the phase-A minimum (wkva+hwin+cwin/swin,
    # 3.7MB): all 8 cores share HBM bandwidth, and every extra early byte
    # delays the slowest core's AllGather trigger, which gates everyone.
    # The remaining input DMAs ride the scalar queue BEHIND the bounce-in
    # writes, so they only start flowing once the collective is in flight.
    wkva_sb = consts.tile([128, HC, 640], BF16)
    wkva_r = d["wkva"].rearrange("(k p) m -> p k m", p=128)
    hwin_sb = consts.tile([128, HC, SW], BF16)
    hwin_r = d["hwin"].rearrange("(k p) m -> p k m", p=128)
    for k in range(0, HC, 2):
        nc.sync.dma_start(out=wkva_sb[:, k:k + 2, :],
                          in_=wkva_r[:, k:k + 2, :])
        nc.sync.dma_start(out=hwin_sb[:, k:k + 2, :],
                          in_=hwin_r[:, k:k + 2, :])
    cwin_sb = consts.tile([128, SW], BF16)
    nc.sync.dma_start(out=cwin_sb[:], in_=d["cwin"])
    swin_sb = consts.tile([128, SW], BF16)
    nc.sync.dma_start(out=swin_sb[:], in_=d["swin"])
    # tiles allocated now, DMAs emitted after the bounce-in writes
    wq_sb = consts.tile([128, HC, 384], BF16)
    ht01 = [htp.tile([128, HC, 512], BF16, tag="ht", bufs=2, name=f"ht{c}")
            for c in range(2)]
    cos_sb = consts.tile([128, S], BF16)
    sin_sb = consts.tile([128, S], BF16)
    wkvb_sb = consts.tile([128, 4, 384], BF16)
    wo_sb = consts.tile([128, HPC, HID], BF16)

    # ---- small constants -------------------------------------------------
    ones_sb = consts.tile([128, 1], BF16)
    nc.vector.memset(ones_sb[:], 1.0)
    ones_row = consts.tile([1, 128], BF16)
    nc.vector.memset(ones_row[:], 1.0)
    # preload the Sqrt ACT table while the ACT queue is idle (the rmsnorm
    # sqrt is on the collective-trigger critical path)
    sq_warm = consts.tile([1, 1], F32)
    nc.vector.memset(sq_warm[:], 1.0)
    sq_warm2 = consts.tile([1, 1], F32)
    nc.scalar.activation(sq_warm2[:], sq_warm[:], AF.Sqrt)
    nexpb = consts.tile([128, 1], F32)
    nc.vector.memset(nexpb[:], -EXPB)
    epsb = consts.tile([1, 1], F32)
    nc.vector.memset(epsb[:], EPS)
    # triangular causal mask for diagonal 128x128 tiles of scoresT[k, q]:
    # keep (0.0) where q_local >= k_local, else NEG
    trimask = consts.tile([128, 128], F32)
    nc.gpsimd.memset(trimask[:], 0.0)
    nc.gpsimd.affine_select(out=trimask[:], in_=trimask[:],
                            compare_op=ALU.is_ge, fill=NEG, base=0,
                            pattern=[[1, 128]], channel_multiplier=-1)

    # ---- persistent activations -----------------------------------------
    # head 1 uses swapped halves: rope rows 0:64, nope rows 64:128
    qT = [big.tile([128, S], BF16, tag=f"qT{h}", name=f"qT{h}")
          for h in range(HPC)]
    kT = [big.tile([128, S], BF16, tag=f"kT{h}", name=f"kT{h}")
          for h in range(HPC)]
    v_sb = big.tile([128, S // 128, HPC * VD], BF16, tag="v")
    ckv_sb = big.tile([128, 4, S], BF16, tag="ckv")

    # =====================================================================
    # PHASE A: sharded kv_a projection [640, SW] + rmsnorm + k-rope,
    # AllGather of the normalized latent + roped k_pe across 8 cores.
    # =====================================================================
    pk_tags = ["sc", "pv", "dn", "po", "sc"]
    pk = [ps.tile([128, SW], F32, tag=pk_tags[m], name=f"pk{m}")
          for m in range(5)]
    sqt = work.tile([128, 4, SW], BF16, tag="sq", bufs=1)
    stg = big.tile([128, 5, SW], BF16, tag="stg")
    # k-major: step k consumes wkva/hwin chunk k right after its DMA lands
    for k in range(HC):
        for m in range(5):
            nc.tensor.matmul(pk[m][:], wkva_sb[:, k, m * 128:(m + 1) * 128],
                             hwin_sb[:, k, :], start=(k == 0),
                             stop=(k == HC - 1))
    for m in range(4):
        nc.vector.tensor_copy(stg[:, m, :], pk[m][:])
        nc.vector.tensor_tensor(sqt[:, m, :], pk[m][:], stg[:, m, :],
                                ALU.mult)
    # k_pe rope: stg[0:64,4] = x'*cos + y*sin  (x' rows 0:64, y 64:128)
    t2a = work.tile([64, SW], F32, tag="t2a", bufs=1)
    nc.vector.tensor_tensor(t2a[:], pk[4][64:128, :],
                            swin_sb[64:128, :], ALU.mult)
    nc.vector.tensor_tensor(stg[0:64, 4, :], pk[4][0:64, :],
                            cwin_sb[0:64, :], ALU.mult)
    nc.vector.tensor_tensor(stg[0:64, 4, :], stg[0:64, 4, :],
                            t2a[:], ALU.add)

    # rms statistics: sum of squares over the 512 latent rows via PE
    ssum = ps.tile([1, SW], F32, tag="dn", name="ssum")
    for m in range(4):
        nc.tensor.matmul(ssum[:], ones_sb[:], sqt[:, m, :], start=(m == 0),
                         stop=(m == 3))
    rsq = work.tile([1, SW], F32, tag="rsq", bufs=1)
    nc.scalar.activation(rsq[:], ssum[:], AF.Sqrt, bias=epsb[:],
                         scale=1.0 / KLR)
    # preload the EXP table now -- the ACT queue stalls on the bounce /
    # readback DMA deps below, and phase D needs the table immediately
    wtile = work.tile([1, 1], F32, tag="wtile", bufs=1)
    nc.scalar.activation(wtile[:], epsb[:], AF.Exp)
    rs = work.tile([1, SW], F32, tag="rs", bufs=1)
    nc.vector.reciprocal(rs[:], rsq[:])
    rs_bf = work.tile([1, SW], BF16, tag="rs_bf", bufs=1)
    nc.vector.tensor_copy(rs_bf[:], rs[:])

    # bounce to DRAM (UNNORMALIZED latent + roped kpe + the rmsnorm scale
    # row), AllGather, read back.  Normalization is applied post-gather so
    # the big latent rows can ship while the stats chain still runs.  The
    # bounce/readback DMAs ride the scalar HWDGE queue: everything behind
    # them there (v copies, exps) is needed only after the gather anyway.
    bin_ = dram.tile([577, SW], BF16, tag="bin")
    bout = dram.tile([NCORES, 577, SW], BF16, tag="bout", addr_space="Shared")
    for m in range(4):
        nc.scalar.dma_start(out=bin_[m * 128:(m + 1) * 128, :],
                            in_=stg[:, m, :])
    nc.scalar.dma_start(out=bin_[512:576, :], in_=stg[0:64, 4, :])
    nc.scalar.dma_start(out=bin_[576:577, :], in_=rs_bf[:])
    nc.gpsimd.collective_compute(
        "AllGather", ALU.bypass,
        replica_groups=[list(range(NCORES))],
        ins=[bin_[:].opt()], outs=[bout[:].opt()])
    # deferred B-feed weight/activation DMAs (scalar queue, behind bounce)
    wq_r = d["wq"].rearrange("(k p) m -> p k m", p=128)
    for kp in range(4):
        nc.scalar.dma_start(out=wq_sb[:, 4 * kp:4 * kp + 4, :],
                            in_=wq_r[:, 4 * kp:4 * kp + 4, :])
    for c in range(2):
        for kp in range(HC // 2):
            nc.scalar.dma_start(
                out=ht01[c][:, 2 * kp:2 * kp + 2, :],
                in_=d["hT"][256 * kp:256 * (kp + 1),
                            c * 512:(c + 1) * 512].rearrange(
                    "(k p) m -> p k m", p=128))
    for i in range(2):
        cs = slice(i * 1024, (i + 1) * 1024)
        nc.scalar.dma_start(out=cos_sb[:, cs], in_=d["cosd"][:, cs])
        nc.scalar.dma_start(out=sin_sb[:, cs], in_=d["sind"][:, cs])
    nc.scalar.dma_start(out=wkvb_sb[:],
                        in_=d["wkvb"].rearrange("(k p) m -> p k m", p=128))
    wo_r = d["wo"].rearrange("(h p) n -> p h n", p=128)
    for h in range(HPC):
        nc.scalar.dma_start(out=wo_sb[:, h, :], in_=wo_r[:, h, :])
    # readback: scale row first (feeds the PE broadcast), then latent tiles
    rs_all = work.tile([1, S], BF16, tag="rs_all", bufs=1)
    nc.scalar.dma_start(
        out=rs_all[:].rearrange("p (c m) -> p c m", m=SW),
        in_=bout[:, 576:577, :].rearrange("c p m -> p c m"))
    rnorm = work.tile([128, 4, 512], BF16, tag="rnorm", bufs=1)
    for i in range(4):
        rn_ps = ps.tile([128, 512], F32, tag="po", name=f"rn_ps{i}")
        nc.tensor.matmul(rn_ps[:], ones_row[:],
                         rs_all[0:1, i * 512:(i + 1) * 512],
                         start=True, stop=True)
        nc.vector.tensor_copy(rnorm[:, i, :], rn_ps[:])
    for half in range(2):
        hs = slice(half * (NCORES // 2), (half + 1) * (NCORES // 2))
        for t in range(4):
            src = bout[:, t * 128:(t + 1) * 128, :]
            nc.scalar.dma_start(
                out=ckv_sb[:, t, half * (S // 2):(half + 1) * (S // 2)]
                    .rearrange("p (c m) -> p c m", m=SW),
                in_=src[hs].rearrange("c p m -> p c m"))
    rope_src = bout[:, 512:576, :].rearrange("c p m -> p c m")
    nc.scalar.dma_start(
        out=kT[0][64:128, :].rearrange("p (c m) -> p c m", m=SW),
        in_=rope_src)
    nc.scalar.dma_start(
        out=kT[1][0:64, :].rearrange("p (c m) -> p c m", m=SW),
        in_=rope_src)
    # normalize the gathered latent in place (per-seq rmsnorm scale)
    rn_flat = bass.AP(tensor=rnorm.tensor, offset=rnorm.offset,
                      ap=[rnorm.ap[0], [1, S]])
    for t in range(4):
        nc.vector.tensor_tensor(ckv_sb[:, t, :], ckv_sb[:, t, :],
                                rn_flat, ALU.mult)

    # =====================================================================
    # PHASE B: q projection (3 blocks: [nope|x']_h0, [x'|nope]_h1,
    # [y_h1|y_h0]) + rope epilogues.
    # =====================================================================
    pq_tags = ["sc", "pv", "dn"]
    for c in range(4):
        cs = slice(c * 512, (c + 1) * 512)
        if c < 2:
            ht_r = ht01[c]
        else:
            ht_r = htp.tile([128, HC, 512], BF16, tag="ht", bufs=2,
                            name=f"ht{c}")
            for kp in range(HC // 2):
                nc.sync.dma_start(
                    out=ht_r[:, 2 * kp:2 * kp + 2, :],
                    in_=d["hT"][256 * kp:256 * (kp + 1), cs].rearrange(
                        "(k p) m -> p k m", p=128))
        pq = [ps.tile([128, 512], F32, tag=pq_tags[m], name=f"pq{m}_{c}")
              for m in range(3)]
        # k-major: consume ht chunk k as soon as its DMA lands
        for k in range(HC):
            for m in range(3):
                nc.tensor.matmul(pq[m][:], wq_sb[:, k, m * 128:(m + 1) * 128],
                                 ht_r[:, k, :], start=(k == 0),
                                 stop=(k == HC - 1))
        t2 = work.tile([128, 512], F32, tag="t2", name=f"t2_{c}")
        # h0: nope rows 0:64, rope rows 64:128
        nc.vector.tensor_copy(qT[0][0:64, cs], pq[0][0:64, :])
        nc.vector.tensor_tensor(qT[0][64:128, cs], pq[0][64:128, :],
                                cos_sb[64:128, cs], ALU.mult)
        nc.vector.tensor_tensor(t2[64:128, :], pq[2][64:128, :],
                                sin_sb[64:128, cs], ALU.mult)
        nc.vector.tensor_tensor(qT[0][64:128, cs], qT[0][64:128, cs],
                                t2[64:128, :], ALU.add)
        # h1 (swapped): rope rows 0:64, nope rows 64:128
        nc.vector.tensor_copy(qT[1][64:128, cs], pq[1][64:128, :])
        nc.vector.tensor_tensor(qT[1][0:64, cs], pq[1][0:64, :],
                                cos_sb[0:64, cs], ALU.mult)
        nc.vector.tensor_tensor(t2[0:64, :], pq[2][0:64, :],
                                sin_sb[0:64, cs], ALU.mult)
        nc.vector.tensor_tensor(qT[1][0:64, cs], qT[1][0:64, cs],
                                t2[0:64, :], ALU.add)

    # =====================================================================
    # PHASE C: kv_b -- k_nope column-major into kT, v row-major into v_sb.
    # =====================================================================
    for c in range(4):
        cs = slice(c * 512, (c + 1) * 512)
        pn = ps.tile([128, 512], F32, tag="sc", name=f"pn{c}")
        for t in range(4):
            nc.tensor.matmul(pn[:], wkvb_sb[:, t, 0:128], ckv_sb[:, t, cs],
                             start=(t == 0), stop=(t == 3))
        nc.vector.tensor_copy(kT[0][0:64, cs], pn[0:64, :])
        nc.vector.tensor_copy(kT[1][64:128, cs], pn[64:128, :])
    for s16 in range(16):
        pvv = ps.tile([128, HPC * VD], F32, tag="pv", name=f"pvv{s16}")
        for t in range(4):
            nc.tensor.matmul(pvv[:], ckv_sb[:, t, s16 * 128:(s16 + 1) * 128],
                             wkvb_sb[:, t, 128:384], start=(t == 0),
                             stop=(t == 3))
        nc.scalar.copy(v_sb[:, s16, :], pvv[:])

    # =====================================================================
    # PHASE D: attention superblocks (512 queries each) + interleaved
    # o_proj of the previous superblock.  Deferred PE work (softmax
    # normalize tails, o_proj tiles) is popped 2 items per kt-iteration,
    # gated by a minimum tick so slow DVE latencies (the reciprocal) never
    # stall the in-order PE queue.
    # =====================================================================
    deferred = []   # list of (min_tick, fn), FIFO
    tick = [0]

    def pop_deferred():
        n = 0
        while deferred and deferred[0][0] <= tick[0] and n < 2:
            deferred.pop(0)[1]()
            n += 1

    def make_oproj_jobs(Bq, a0, a1):
        jobs = []
        for t in range(4):
            ot = outp.tile([128, 4, 512], BF16, tag="ot", bufs=2,
                           name=f"ot{Bq}_{t}")
            for n in range(4):
                def job(Bq=Bq, t=t, n=n, ot=ot, a0=a0, a1=a1):
                    po = ps.tile([128, 512], F32, tag="po",
                                 name=f"po{Bq}_{t}_{n}")
                    nc.tensor.matmul(po[:], a0[:, t * 128:(t + 1) * 128],
                                     wo_sb[:, 0, n * 512:(n + 1) * 512],
                                     start=True, stop=False)
                    nc.tensor.matmul(po[:], a1[:, t * 128:(t + 1) * 128],
                                     wo_sb[:, 1, n * 512:(n + 1) * 512],
                                     start=False, stop=True)
                    if n % 2 == 0:
                        nc.vector.tensor_copy(ot[:, n, :], po[:])
                    else:
                        nc.scalar.copy(ot[:, n, :], po[:])
                    if n == 3:
                        nc.sync.dma_start(
                            out=d["o"][(4 * Bq + t) * 128:
                                       (4 * Bq + t + 1) * 128, :],
                            in_=ot[:])
                jobs.append(job)
        return jobs

    at_tiles = [None, None]
    for Bq in range(4):
        nkt = 4 * (Bq + 1)
        for h in range(HPC):
            pa = ps.tile([128, 512], F32, tag="pv", name=f"pa{Bq}_{h}")
            dnm = ps.tile([1, 512], F32, tag="dn", name=f"dn{Bq}_{h}")

            def emit_score(kt, h=h, Bq=Bq):
                j = kt - 4 * Bq
                off = j * 128 if j >= 0 else 0
                w = 512 - off
                sct = ps.tile([128, 512], F32, tag="sc",
                              name=f"sc{Bq}_{h}_{kt}")
                nc.tensor.matmul(
                    sct[:, 0:w], kT[h][:, kt * 128:(kt + 1) * 128],
                    qT[h][:, Bq * 512 + off:(Bq + 1) * 512],
                    start=True, stop=True)
                return sct, off, w

            cur = emit_score(0)
            for kt in range(nkt):
                sct, off, w = cur
                if kt + 1 < nkt:
                    cur = emit_score(kt + 1)
                if kt - 4 * Bq >= 0:  # diagonal tile: triangular mask
                    nc.vector.tensor_tensor(sct[:, 0:128], sct[:, 0:128],
                                            trimask[:], ALU.add)
                ex = work.tile([128, 512], BF16, tag="expT", bufs=3,
                               name=f"ex{Bq}_{h}_{kt}")
                nc.scalar.activation(ex[:, off:512], sct[:, 0:w], AF.Exp,
                                     bias=nexpb[:], scale=1.0)
                # deferred work popped between exp and ones/PV emission:
                # its matmuls give the PE filler while exp(kt) is in
                # flight, and its copies queue on DVE after the mask
                pop_deferred()
                nc.tensor.matmul(dnm[0:1, off:512], ones_sb[:],
                                 ex[:, off:512], start=(kt == 0),
                                 stop=(kt == nkt - 1))
                nc.tensor.matmul(pa[:, off:512],
                                 v_sb[:, kt, h * VD:(h + 1) * VD],
                                 ex[:, off:512], start=(kt == 0),
                                 stop=(kt == nkt - 1))
                tick[0] += 1

            # softmax normalize: reciprocal of the denominator row (slow
            # DVE op) runs now; the PE broadcast + scale are deferred so
            # the PE queue never waits on the reciprocal's latency.
            rr = work.tile([1, 512], F32, tag="rr", name=f"rr{Bq}_{h}")
            nc.vector.reciprocal(rr[:], dnm[:])
            rr_bf = work.tile([1, 512], BF16, tag="rr_bf",
                              name=f"rr_bf{Bq}_{h}")
            nc.vector.tensor_copy(rr_bf[:], rr[:])
            at = work.tile([128, 512], BF16, tag=f"at{h}", bufs=2,
                           name=f"at{Bq}_{h}")

            def norm_tail(Bq=Bq, h=h, pa=pa, rr_bf=rr_bf, at=at):
                rb_ps = ps.tile([128, 512], F32, tag="po",
                                name=f"rb_ps{Bq}_{h}")
                nc.tensor.matmul(rb_ps[:], ones_row[:], rr_bf[:],
                                 start=True, stop=True)
                rbc2 = work.tile([128, 512], BF16, tag="rbc2",
                                 name=f"rbc2{Bq}_{h}")
                nc.vector.tensor_copy(rbc2[:], rb_ps[:])
                nc.vector.tensor_tensor(at[:], pa[:], rbc2[:], ALU.mult)
            deferred.append((tick[0] + 5, norm_tail))
            at_tiles[h] = at
        jobs = make_oproj_jobs(Bq, at_tiles[0], at_tiles[1])
        deferred.extend((tick[0] + 7, j) for j in jobs)
    while deferred:
        deferred.pop(0)[1]()


# =========================================================================
# host side
# =========================================================================
_perm1 = np.concatenate([np.arange(0, ROPE, 2), np.arange(1, ROPE, 2)])
_perm2 = np.concatenate([np.arange(1, ROPE, 2), np.arange(0, ROPE, 2)])
_sgn2 = np.concatenate([-np.ones(ROPE // 2), np.ones(ROPE // 2)]).astype(np.float32)


def _host_prep(inputs):
    hidden = np.ascontiguousarray(np.asarray(inputs["hidden_states"],
                                             dtype=np.float32)[0])
    cos = np.asarray(inputs["cos"], dtype=np.float32)[0]
    sin = np.asarray(inputs["sin"], dtype=np.float32)[0]
    w_q = np.asarray(inputs["w_q"], dtype=np.float32)
    w_kv_a = np.asarray(inputs["w_kv_a"], dtype=np.float32)
    ln_w = np.asarray(inputs["kv_a_ln_w"], dtype=np.float32)
    w_kv_b = np.asarray(inputs["w_kv_b"], dtype=np.float32)
    w_o = np.asarray(inputs["w_o"], dtype=np.float32)

    hT = np.ascontiguousarray(hidden.T)
    cosT, sinT = cos.T, sin.T
    cosd = np.ascontiguousarray(np.concatenate([cosT, cosT], axis=0))
    sind = np.ascontiguousarray(np.concatenate([sinT, sinT], axis=0))

    kpe_cols = w_kv_a[:, KLR:]
    wkva_mod = np.ascontiguousarray(np.concatenate(
        [w_kv_a[:, :KLR], kpe_cols[:, _perm1], kpe_cols[:, _perm2] * _sgn2[None, :]],
        axis=1))
    wkvb_all = w_kv_b * ln_w[:, None]

    bf = ml_dtypes.bfloat16
    hT_bf = hT.astype(bf)
    wkva_bf = wkva_mod.astype(bf)
    cosd_bf = cosd.astype(bf)
    sind_bf = sind.astype(bf)

    in_maps = []
    for c in range(NCORES):
        heads = [HPC * c + i for i in range(HPC)]
        win = slice(c * SW, (c + 1) * SW)
        h0, h1 = heads
        wq_h0 = w_q[:, h0 * QD:(h0 + 1) * QD]
        wq_h1 = w_q[:, h1 * QD:(h1 + 1) * QD]
        b0 = np.concatenate([wq_h0[:, :NOPE], wq_h0[:, NOPE:][:, _perm1]],
                            axis=1)
        b1 = np.concatenate([wq_h1[:, NOPE:][:, _perm1], wq_h1[:, :NOPE]],
                            axis=1)
        b2 = np.concatenate([wq_h1[:, NOPE:][:, _perm2] * _sgn2[None, :],
                             wq_h0[:, NOPE:][:, _perm2] * _sgn2[None, :]],
                            axis=1)
        wq_mod = np.ascontiguousarray(
            np.concatenate([b0, b1, b2], axis=1) * SCALE)

        nope_b = [wkvb_all[:, h * (NOPE + VD):h * (NOPE + VD) + NOPE]
                  for h in heads]
        v_b = [wkvb_all[:, h * (NOPE + VD) + NOPE:(h + 1) * (NOPE + VD)]
               for h in heads]
        wkvb_mod = np.ascontiguousarray(np.concatenate(nope_b + v_b, axis=1))

        wo_mod = np.ascontiguousarray(w_o[h0 * VD:(h1 + 1) * VD, :])

        cwin = np.ascontiguousarray(np.concatenate(
            [cosT[:, win], cosT[:, win]], axis=0))
        swin = np.ascontiguousarray(np.concatenate(
            [sinT[:, win], sinT[:, win]], axis=0))

        in_maps.append({
            "hT": hT_bf,
            "hwin": np.ascontiguousarray(hT[:, win]).astype(bf),
            "wq": wq_mod.astype(bf),
            "wkva": wkva_bf,
            "wkvb": wkvb_mod.astype(bf),
            "wo": wo_mod.astype(bf),
            "cosd": cosd_bf, "sind": sind_bf,
            "cwin": cwin.astype(bf), "swin": swin.astype(bf),
        })
    return in_maps


def _install_ntff_hook():
    """Make trace=True work under axon (antenv.axon_hooks is absent in this
    image; back it with trn_agent_boot's ctypes hook)."""
    try:
        import antenv
        if "antenv.axon_hooks" in sys.modules:
            return
        from trn_agent_boot.trn_boot import _ntff_profile_via_ctypes
        hook = _ntff_profile_via_ctypes("/opt/axon/libaxon_pjrt.so")
        mod = types.ModuleType("antenv.axon_hooks")
        mod.get_axon_ntff_profile_hook = lambda: hook
        mod.set_axon_ntff_profile_hook = lambda h: None
        sys.modules["antenv.axon_hooks"] = mod
        antenv.axon_hooks = mod
    except Exception:
        pass


_nc_cache = None
last_results = None


def kernel(**inputs):
    global _nc_cache, last_results
    _install_ntff_hook()
    if _nc_cache is None:
        _nc_cache = build_nc()
    in_maps = _host_prep(inputs)
    trace = bool(os.environ.get("BASS_TRACE"))
    res = bass_utils.run_bass_kernel_spmd(
        _nc_cache, in_maps, core_ids=list(range(NCORES)), trace=trace)
    last_results = res
    total = res.results[0]["o"].astype(np.float32)
    for c in range(1, NCORES):
        total = total + res.results[c]["o"]
    return total.reshape(1, S, HID)
